# revision 38
# baseline (speedup 1.0000x reference)
"""Trainium2 Bass kernel for nn_EquiConv2d (equirectangular deformable conv).

Structure (v2 — x-interp prefused on vector engines):
  * off_y is longitude-invariant: each (tap k, row h) samples a fixed input
    row-pair (iy0, iy0+1) with constant y-fractions -> row-pair tiles F
    ([128 = 2rows x 64ch, 1024 = row duplicated for circular reads]).
  * off_x is longitude-invariant up to the 2*pi wrap: sampling along a row is
    a circular shift s0(k,h) plus a constant x-fraction fr.
  * v1 used 2 matmuls per tap (x0/x1 corners, scales folded in stationary).
    v2 prefuses the x-interp on DVE/Pool/Act:
        z = win_a + c*win_b   (c = min(fr,1-fr)/max(..) <= 1)
    so each tap is ONE matmul [128-contraction, 512-free] with the max corner
    weight folded into the per-(row,tap) stationary -> 9 matmuls/row.
  * Seam zero-padding semantics (reference treats x outside [0,512) as zero):
    a "G-type" window equals the F window except column 512 of F must read 0.
    Instead of staging a second zeroed tile: prefused taps patch the single
    affected z column ([128,1] copy / tensor_scalar); single-slot taps split
    their matmul around the affected output column (PSUM accumulation makes
    the skipped column exactly the required zero contribution).
  * fp32 oddities: tap (7,255) dead; tap (1,1) antipode handled by 3 extra
    data-driven matmul slots (active only on cores owning global row 1).

Sharding: 8 cores = 2 batches x 4 bands of 64 output rows.
"""

import math

import numpy as np

# ----------------------------------------------------------------------------
# problem constants
B, C, H, W = 2, 64, 256, 512
O, KH, KW = 64, 3, 3
K = KH * KW
NCORES = 8
NROW = 64            # output rows per core
MAXZ = 5             # prefused doubles per row (engine capacity bound)
NACT = 2             # prefused builds whose mult half runs on Act
NTAP = 10            # stationary slots per row (9 taps + un-prefused extras)
NSPEC = 3            # special (antipode) slots, accumulated into local row 1
RING = 16            # staged row-pair ring slots
PF = 4               # staging prefetch lead (rows)
SLOTW = 1024         # F columns per ring slot (row-pair duplicated)
SKIP_TOL = 1e-4      # drop corner slots with |weight| below this

_CACHE = {}


# ----------------------------------------------------------------------------
# host-side geometry tables (must replicate reference fp32 semantics exactly)

def _compute_offsets_jax():
    """Bit-exact replica of reference.equi_offsets on jax CPU."""
    import jax
    import jax.numpy as jnp
    cpu = jax.devices("cpu")[0]
    with jax.default_device(cpu):
        dtype = jnp.float32
        pano_H, pano_W, kH, kW = H, W, KH, KW
        Kk = kH * kW
        u = jnp.arange(pano_W, dtype=dtype)
        v = jnp.arange(pano_H, dtype=dtype)
        phi = (u - pano_W / 2.0) / pano_W * (2.0 * math.pi)
        theta = -(v - pano_H / 2.0) / pano_H * math.pi
        cp, sp = jnp.cos(phi), jnp.sin(phi)
        z, one = jnp.zeros_like(cp), jnp.ones_like(cp)
        Ry = jnp.stack([jnp.stack([cp, z, sp], -1),
                        jnp.stack([z, one, z], -1),
                        jnp.stack([-sp, z, cp], -1)], -2)
        ct, st = jnp.cos(theta), jnp.sin(theta)
        zh, oh = jnp.zeros_like(ct), jnp.ones_like(ct)
        Rx = jnp.stack([jnp.stack([oh, zh, zh], -1),
                        jnp.stack([zh, ct, -st], -1),
                        jnp.stack([zh, st, ct], -1)], -2)
        ROT = jnp.einsum('wij,hjk->hwik', Ry, Rx)
        fov_w = kW * (2.0 * math.pi / pano_W)
        focal = (kW / 2.0) / math.tan(fov_w / 2.0)
        hg = (jnp.arange(kH, dtype=dtype)[:, None] + 0.5 - kH / 2.0)
        wg = (jnp.arange(kW, dtype=dtype)[None, :] + 0.5 - kW / 2.0)
        hg = jnp.broadcast_to(hg, (kH, kW)).reshape(Kk)
        wg = jnp.broadcast_to(wg, (kH, kW)).reshape(Kk)
        rays0 = jnp.stack([wg / focal, hg / focal, jnp.ones(Kk, dtype)], 0)
        rays0 = rays0 / jnp.linalg.norm(rays0, axis=0, keepdims=True)
        rays = jnp.einsum('hwik,kn->hwin', ROT, rays0)
        phi2 = jnp.arctan2(rays[..., 0, :], rays[..., 2, :])
        th2 = jnp.arcsin(jnp.clip(rays[..., 1, :], -1.0, 1.0))
        x = pano_W / (2.0 * math.pi) * phi2 + pano_W / 2.0
        y = pano_H / math.pi * th2 + pano_H / 2.0
        off_x = x - (wg[None, None, :] + u[None, :, None])
        off_y = y - (hg[None, None, :] + v[:, None, None])
        return (np.asarray(jnp.transpose(off_y, (2, 0, 1))),
                np.asarray(jnp.transpose(off_x, (2, 0, 1))))


def _build_tap_tables():
    off_y, off_x = _compute_offsets_jax()
    ky = np.repeat(np.arange(KH), KW).astype(np.float32)
    kx = np.tile(np.arange(KW), KH).astype(np.float32)
    base_x = (np.arange(W, dtype=np.float32) - np.float32(1))
    base_y = (np.arange(H, dtype=np.float32) - np.float32(1))
    px = (base_x[None, None, :] + kx[:, None, None] + off_x).astype(np.float32)
    py = (base_y[None, :, None] + ky[:, None, None] + off_y).astype(np.float32)
    pyc = py[:, :, 0]
    assert np.all(py == pyc[:, :, None]), "off_y not longitude-invariant"

    iy0 = np.floor(pyc).astype(np.int64)
    wy1 = (pyc - np.floor(pyc)).astype(np.float64)
    v0 = (iy0 >= 0) & (iy0 < H)
    v1 = (iy0 + 1 >= 0) & (iy0 + 1 < H)
    cy0 = np.where(v0, 1.0 - wy1, 0.0)
    cy1 = np.where(v1, wy1, 0.0)

    Draw = np.mod((px.astype(np.float64) - np.arange(W)[None, None, :]), 512.0)
    ang = Draw / 512.0 * 2 * np.pi
    mean = np.mod(np.angle(np.exp(1j * ang).mean(axis=2)) / (2 * np.pi) * 512.0,
                  512.0)
    resid = np.mod(Draw - mean[:, :, None] + 256.0, 512.0) - 256.0
    D = mean + np.median(resid, axis=2)
    s0 = np.mod(np.floor(D), 512).astype(np.int64)
    frac = D - np.floor(D)

    special = np.zeros((K, H), dtype=bool)
    special[1, 1] = True
    dead = (cy0 == 0.0) & (cy1 == 0.0)

    Ddev = np.abs(np.mod(Draw - D[:, :, None] + 256.0, 512.0) - 256.0)
    dev = Ddev.max(axis=2)
    bad = (dev > 5e-4) & ~special & ~dead
    assert not bad.any(), f"unrepresentable taps: {np.argwhere(bad)}"

    def ref_coefs(p):
        x0 = math.floor(p)
        fr = p - x0
        out = {}
        for ix, wt in ((x0, 1.0 - fr), (x0 + 1, fr)):
            if 0 <= ix < W and wt != 0.0:
                out[ix] = out.get(ix, 0.0) + wt
        return out

    # seam variant selection: decided by the exact fp32 px at the wrap column
    slot0_useG = np.zeros((K, H), dtype=bool)
    slot1_useF = np.zeros((K, H), dtype=bool)
    for k in range(K):
        for h in range(H):
            if special[k, h] or dead[k, h]:
                continue
            s = int(s0[k, h]); fr = frac[k, h]
            if s >= 1:
                w0 = (512 - s) % 512
                rc = ref_coefs(float(px[k, h, w0]))
                slot0_useG[k, h] = (abs(rc.get(0, 0.0))
                                    < abs(rc.get(0, 0.0) - (1 - fr)))
            w1 = (511 - s) % 512
            rc = ref_coefs(float(px[k, h, w1]))
            slot1_useF[k, h] = (abs(rc.get(0, 0.0) - fr)
                                < abs(rc.get(0, 0.0)))

    # special tap (1,1): per-column coefficients on F offsets 255..257
    pxs = px[1, 1, :].astype(np.float64)
    Gam = np.zeros((3, W), dtype=np.float64)
    for w in range(W):
        p = pxs[w]
        x0 = math.floor(p)
        fr = p - x0
        for ix, wt in ((x0, 1.0 - fr), (x0 + 1, fr)):
            if 0 <= ix < W and wt != 0.0:
                found = False
                for jj in range(3):
                    if (255 + jj + w) % 512 == ix % 512:
                        Gam[jj, w] += wt
                        found = True
                        break
                assert found, (w, p, ix)

    return dict(iy0=iy0, cy0=cy0, cy1=cy1, s0=s0, frac=frac,
                slot0_useG=slot0_useG, slot1_useF=slot1_useF,
                special=special, dead=dead, Gam=Gam)


# ----------------------------------------------------------------------------
# uniform SPMD schedule (events = staged row-pairs per band)

def _build_schedule(tt):
    blocks = []
    for blk in range(4):
        h0 = blk * NROW
        ev_of, events, first_use = {}, [], []
        need = np.zeros((NROW, K), np.int64)
        for lh in range(NROW):
            for k in range(K):
                r = int(np.clip(tt['iy0'][k, h0 + lh], 0, 255))
                if r not in ev_of:
                    ev_of[r] = len(events)
                    events.append(r)
                    first_use.append(lh)
                need[lh, k] = ev_of[r]
        blocks.append(dict(events=events, first_use=first_use, need=need))

    E = max(len(b['events']) for b in blocks)
    for b in blocks:
        while len(b['events']) < E:
            b['events'].append(b['events'][-1])
    return blocks, E


# ----------------------------------------------------------------------------
# per-row tap plan: windows, prefusion, patches, splits, engine assignment

def _build_plan(tt, blocks):
    """plans[blk][lh] = ordered list of tap items (full-width first):
       ('z', ev, f0a, f0b, c, patch_a, patch_b)   prefused double
       ('s', ev, f0, ws)                          single; ws=None -> full
    At most MAXZ doubles per row are prefused (engine capacity); the rest
    (those with the most seam patches, which become ~free PE splits) are
    emitted as two single slots.
    Also returns scales[blk][lh, ti, :] (stationary scale [128]) and
    kmaps[blk][lh][ti] (source tap k per stationary slot).
    """
    plans, scales, kmaps = [], [], []
    for blk in range(4):
        need = blocks[blk]['need']
        rows, krows = [], []
        sc = np.zeros((NROW, NTAP, 128), np.float64)
        for lh in range(NROW):
            h = blk * NROW + lh
            doubles, singles = [], []
            for k in range(K):
                if tt['dead'][k, h] or tt['special'][k, h]:
                    continue
                ev = int(need[lh, k])
                s = int(tt['s0'][k, h])
                fr = float(tt['frac'][k, h])
                c0, c1 = float(tt['cy0'][k, h]), float(tt['cy1'][k, h])
                f0a, f0b = s, s + 1
                a_fp = bool(tt['slot0_useG'][k, h]) and s >= 1 and f0a >= 1
                b_fp = (not bool(tt['slot1_useF'][k, h])) and f0b >= 1
                e0 = 1.0 - fr >= SKIP_TOL
                e1 = fr >= SKIP_TOL
                if e0 and e1:
                    doubles.append((k, ev, f0a, f0b, fr, a_fp, b_fp, c0, c1))
                elif e0 or e1:
                    wt, f0, fp = ((1.0 - fr), f0a, a_fp) if e0 \
                        else (fr, f0b, b_fp)
                    ws = 512 - f0 if fp else None
                    singles.append(((('s', ev, f0, ws), c0 * wt, c1 * wt), k))
            # prefuse the doubles with the fewest patches; un-prefuse rest.
            # build-engine split: first NACT prefused get Act mults ('a'),
            # the rest DVE ts-mults ('v').  DVE-built z's are emitted first
            # in the matmul order (ready earliest).
            doubles.sort(key=lambda d: int(d[5]) + int(d[6]))
            za, zv, zrest = [], [], []
            for di, (k, ev, f0a, f0b, fr, a_fp, b_fp, c0, c1) in \
                    enumerate(doubles):
                if di < MAXZ:
                    if fr <= 0.5:
                        aw, cc = 1.0 - fr, fr / (1.0 - fr)
                        wa_f0, wb_f0, wa_fp, wb_fp = f0a, f0b, a_fp, b_fp
                    else:
                        aw, cc = fr, (1.0 - fr) / fr
                        wa_f0, wb_f0, wa_fp, wb_fp = f0b, f0a, b_fp, a_fp
                    pa = 512 - wa_f0 if wa_fp else None
                    pb = 512 - wb_f0 if wb_fp else None
                    eng = 'a' if di < NACT else 'v'
                    it = ((('z', ev, wa_f0, wb_f0, cc, pa, pb, eng),
                           c0 * aw, c1 * aw), k)
                    (za if eng == 'a' else zv).append(it)
                else:
                    for wt, f0, fp in (((1.0 - fr), f0a, a_fp),
                                       (fr, f0b, b_fp)):
                        ws = 512 - f0 if fp else None
                        it = ((('s', ev, f0, ws), c0 * wt, c1 * wt), k)
                        (zrest if ws is not None else singles).append(it)
            full = zv + za + [it for it in singles if it[0][0][3] is None]
            rest = zrest + [it for it in singles if it[0][0][3] is not None]
            assert full, (blk, lh)
            items = full + rest
            assert len(items) <= NTAP, (blk, lh, len(items))
            row, krow = [], []
            for ti, ((it, s0c, s1c), k) in enumerate(items):
                sc[lh, ti, :64] = s0c
                sc[lh, ti, 64:] = s1c
                row.append(it)
                krow.append(k)
            rows.append(row)
            krows.append(krow)
        plans.append(rows)
        scales.append(sc)
        kmaps.append(krows)

    # beta bias carrier: per row, the first Act-built prefused double.
    # Its activation folds the per-partition beta (solving lt_tap^T beta =
    # bias) in for free; its seam patches become beta-aware [128,1] ops.
    betas = []
    for blk in range(4):
        bt = []
        for lh in range(NROW):
            car = None
            for ti, it in enumerate(plans[blk][lh]):
                if it[0] == 'z' and it[7] == 'a':
                    car = (0, ti)
                    break
            bt.append(car)
        betas.append(bt)
    return plans, scales, kmaps, betas


# ----------------------------------------------------------------------------
# device program

def _emit_section(tc, aps, tiles, pools, blkinfo, plan, betas_row,
                  special_row, shared_dmas):
    """Emit one per-band section (all-static APs)."""
    import concourse.mybir as mybir
    import bass_rust
    nc = tc.nc
    f16 = mybir.dt.float16
    f32 = mybir.dt.float32
    AL = mybir.AluOpType
    ID = mybir.ActivationFunctionType.Identity
    buf, coeft, biast, ltst, betat = tiles
    psp, ltp, zp, zmp, spzp, outp = pools
    xb, outd, lt = aps['xb'], aps['out'], aps['lt']
    first_use = blkinfo['first_use']

    cum = [int(np.searchsorted(np.asarray(first_use), lh, 'right'))
           for lh in range(NROW)]
    tgt = [cum[min(lh + PF, NROW - 1)] for lh in range(NROW)]

    # ring-overwrite feasibility
    E_j = len(first_use)
    ls = [NROW] * E_j
    for e in range(E_j):
        for lh in range(NROW):
            if tgt[lh] > e:
                ls[e] = lh
                break
    lastuse = {}
    need = blkinfo['need']
    for lh in range(NROW):
        for k in range(K):
            lastuse[int(need[lh, k])] = lh
    for e in range(RING, E_j):
        if e - RING in lastuse:
            assert lastuse[e - RING] < ls[e], (e,)

    def stage(e):
        base = (e % RING) * SLOTW
        src = xb[e].rearrange("p c w -> (p c) w")
        nc.sync.dma_start(buf[:, base:base + W], src)
        nc.sync.dma_start(buf[:, base + W:base + 2 * W], src)

    def build_row(lh):
        """z-builds for row lh: mult half on Act ('a' builds, activation
        ~700ns, the carrier folds the per-partition beta bias in for free)
        or DVE ts ('v' builds, ~170ns); adds on DVE, PAIRED two-at-a-time
        via a strided 3-dim in1 AP over buf.  Each pair couples one Act and
        one DVE mult so a slow Act never gates two z's.  Seam patches: ts-
        type on DVE, plain copies on the otherwise-idle Pool."""
        zv = [(ti, it) for ti, it in enumerate(plan[lh])
              if it[0] == 'z' and it[7] == 'v']
        za = [(ti, it) for ti, it in enumerate(plan[lh])
              if it[0] == 'z' and it[7] == 'a']
        carrier = betas_row[lh]
        zs = {}
        # pair one 'a' with one 'v' when possible
        grps = []
        while za or zv:
            g = []
            if za:
                g.append(za.pop(0))
            if zv:
                g.append(zv.pop(0))
            grps.append(g)

        def offa(entry):
            it = entry[1]
            return (it[1] % RING) * SLOTW + it[2]

        for p, grp in enumerate(grps):
            if len(grp) == 2 and offa(grp[0]) > offa(grp[1]):
                grp = [grp[1], grp[0]]
            zpt = zp.tile([128, 2 * W], f16, tag=f"z{p}")
            zmt = zmp.tile([128, 2 * W], f16, tag=f"zm{p}")
            for h, (ti, it) in enumerate(grp):
                _, ev, fa, fb, cc, pa, pb, eng = it
                base = (ev % RING) * SLOTW
                winb = buf[:, base + fb:base + fb + W]
                zmh = zmt[:, h * W:(h + 1) * W]
                if eng == 'a':
                    barg = betat[:, lh:lh + 1] \
                        if (carrier is not None and carrier[1] == ti) else 0.0
                    nc.scalar.activation(zmh, winb, ID, bias=barg,
                                         scale=float(cc))
                else:
                    nc.vector.tensor_scalar(zmh, winb, float(cc), None,
                                            AL.mult)
            if len(grp) == 2 and offa(grp[1]) != offa(grp[0]):
                o0 = offa(grp[0])
                d = offa(grp[1]) - o0
                pair = bass_rust.AP(buf.tensor, buf.offset + o0,
                                    [list(buf.ap[0]), [d, 2], [1, W]])
                zm3 = zmt.rearrange("p (t w) -> p t w", t=2)
                z3 = zpt.rearrange("p (t w) -> p t w", t=2)
                nc.vector.tensor_tensor(z3, zm3, pair, AL.add)
            else:
                for h, (ti, it) in enumerate(grp):
                    base = (it[1] % RING) * SLOTW
                    wina = buf[:, base + it[2]:base + it[2] + W]
                    nc.vector.tensor_tensor(zpt[:, h * W:(h + 1) * W],
                                            zmt[:, h * W:(h + 1) * W],
                                            wina, AL.add)
            for h, (ti, it) in enumerate(grp):
                _, ev, fa, fb, cc, pa, pb, eng = it
                base = (ev % RING) * SLOTW
                iscar = carrier is not None and carrier[1] == ti
                bcol = betat[:, lh:lh + 1]
                if pa is not None:
                    # win_a zero at col pa: z[:,pa] = c*win_b[:,pa] (+beta)
                    if iscar:
                        nc.vector.scalar_tensor_tensor(
                            zpt[:, h * W + pa:h * W + pa + 1],
                            buf[:, base + fb + pa:base + fb + pa + 1],
                            float(cc), bcol, AL.mult, AL.add)
                    else:
                        nc.vector.tensor_scalar(
                            zpt[:, h * W + pa:h * W + pa + 1],
                            buf[:, base + fb + pa:base + fb + pa + 1],
                            float(cc), None, AL.mult)
                if pb is not None:
                    # win_b zero at col pb: z[:,pb] = win_a[:,pb] (+beta)
                    if iscar:
                        nc.vector.tensor_tensor(
                            zpt[:, h * W + pb:h * W + pb + 1],
                            buf[:, base + fa + pb:base + fa + pb + 1],
                            bcol, AL.add)
                    else:
                        nc.gpsimd.tensor_copy(
                            zpt[:, h * W + pb:h * W + pb + 1],
                            buf[:, base + fa + pb:base + fa + pb + 1])
                zs[ti] = zpt[:, h * W:(h + 1) * W]
        return zs

    staged = 0
    zs_d = {}
    for lh in range(NROW):
        while staged < tgt[lh]:
            stage(staged)
            staged += 1
        ltt = ltp.tile([128, NTAP * O], f16, tag="ltt")
        half = NTAP * O // 2
        nc.sync.dma_start(ltt[:, :half], lt[lh][:, :half])
        nc.sync.dma_start(ltt[:, half:], lt[lh][:, half:])
        if lh == 0:
            shared_dmas()
            zs_d[0] = build_row(0)
            if NROW > 1:
                zs_d[1] = build_row(1)
        ps = psp.tile([O, W], f32, tag="ps")

        items = plan[lh]
        # depth-2 software pipeline: z tiles for rows lh..lh+2 are built
        # while the PE works rows lh-2..lh (keeps Act/DVE — and Act's
        # in-order queue behind the out copy — off the PE critical path)
        if lh + 2 < NROW:
            zs_d[lh + 2] = build_row(lh + 2)
        zs = zs_d.pop(lh)

        # matmuls: count instructions first for start/stop flags
        nmm = 0
        for it in items:
            if it[0] == 'z':
                nmm += 1
            else:
                ws = it[3]
                if ws is None:
                    nmm += 1
                else:
                    nmm += int(ws > 0) + int(ws < 511)
        nmm += NSPEC if special_row == lh else 0

        mi = 0
        for ti, it in enumerate(items):
            lts_ap = ltt[:, ti * O:(ti + 1) * O]
            if it[0] == 'z':
                nc.tensor.matmul(ps, lts_ap, zs[ti],
                                 start=(mi == 0), stop=(mi == nmm - 1))
                mi += 1
            else:
                _, ev, f0, ws = it
                base = (ev % RING) * SLOTW
                win = buf[:, base + f0:base + f0 + W]
                if ws is None:
                    nc.tensor.matmul(ps, lts_ap, win,
                                     start=(mi == 0), stop=(mi == nmm - 1))
                    mi += 1
                else:
                    if ws > 0:
                        nc.tensor.matmul(ps[:, 0:ws], lts_ap, win[:, 0:ws],
                                         start=(mi == 0),
                                         stop=(mi == nmm - 1))
                        mi += 1
                    if ws < 511:
                        nc.tensor.matmul(ps[:, ws + 1:W], lts_ap,
                                         win[:, ws + 1:W],
                                         start=(mi == 0),
                                         stop=(mi == nmm - 1))
                        mi += 1

        if special_row == lh:
            sbase = (aps['spec_ev'] % RING) * SLOTW
            for jj in range(NSPEC):
                zt = spzp.tile([128, W], f16, tag="spz")
                nc.vector.tensor_tensor(
                    zt, buf[:, sbase + 255 + jj:sbase + 255 + jj + W],
                    coeft[:, jj * W:(jj + 1) * W], AL.mult)
                nc.tensor.matmul(ps, ltst[:, jj * O:(jj + 1) * O], zt,
                                 start=False, stop=(mi == nmm - 1))
                mi += 1

        ot = outp.tile([O, W], f16, tag="out")
        if betas_row[lh] is not None:
            # bias was injected via the carrier z-build; plain Act copy
            # (GPSIMD cannot read PSUM, and DMA-from-PSUM is unsupported)
            nc.scalar.copy(ot, ps)
        else:
            nc.scalar.activation(ot, ps, ID, bias=biast, scale=1.0)
        nc.sync.dma_start(outd[lh], ot)


def _emit_kernel(tc, aps, blocks, plans, betas, spec_ev):
    import concourse.mybir as mybir
    nc = tc.nc
    f16 = mybir.dt.float16
    f32 = mybir.dt.float32

    with tc.tile_pool(name="bigp", bufs=1) as bigp, \
         tc.tile_pool(name="ltp", bufs=6) as ltp, \
         tc.tile_pool(name="zp", bufs=3) as zp, \
         tc.tile_pool(name="zmp", bufs=3) as zmp, \
         tc.tile_pool(name="spzp", bufs=3) as spzp, \
         tc.tile_pool(name="psp", bufs=6, space="PSUM") as psp, \
         tc.tile_pool(name="outp", bufs=10) as outp:

        buf = bigp.tile([128, RING * SLOTW], f16)
        coeft = bigp.tile([128, NSPEC * W], f16)
        biast = bigp.tile([O, 1], f32)
        ltst = bigp.tile([128, NSPEC * O], f16)
        betat = bigp.tile([128, NROW], f32)

        blkv = nc.values_load(aps['blkid'][0:1, 0:1],
                              min_val=0, max_val=3,
                              skip_runtime_bounds_check=True)

        def shared_dmas():
            # emitted inside each band section AFTER the initial event
            # staging + first lt row so those win the queue-dispatch race
            nc.sync.dma_start(betat, aps['betad'])
            nc.sync.dma_start(biast, aps['biasd'])
            nc.sync.dma_start(ltst, aps['lts'])
            nc.sync.dma_start(coeft, aps['coefr'])

        tiles = (buf, coeft, biast, ltst, betat)
        pools = (psp, ltp, zp, zmp, spzp, outp)
        for j in range(4):
            with tc.If(blkv == j):
                aps['spec_ev'] = spec_ev
                _emit_section(tc, aps, tiles, pools, blocks[j], plans[j],
                              betas[j], special_row=(1 if j == 0 else -1),
                              shared_dmas=shared_dmas)


def _get_compiled():
    """Build tables, schedule, and the Bass program once."""
    if 'prog' in _CACHE:
        return _CACHE['prog']
    import concourse.mybir as mybir
    import concourse.tile as tile
    from concourse import bacc

    tt = _build_tap_tables()
    blocks, E = _build_schedule(tt)
    plans, scales, kmaps, betas = _build_plan(tt, blocks)
    spec_ev = int(blocks[0]['need'][1, 1])

    f16 = mybir.dt.float16
    f32 = mybir.dt.float32
    nc = bacc.Bacc("TRN2", target_bir_lowering=False, debug=False,
                   num_devices=NCORES)
    aps = {
        'xb': nc.dram_tensor("xb", [E, 2, C, W], f16,
                             kind="ExternalInput").ap(),
        'lt': nc.dram_tensor("lt", [NROW, 128, NTAP * O], f16,
                             kind="ExternalInput").ap(),
        'lts': nc.dram_tensor("lts", [128, NSPEC * O], f16,
                              kind="ExternalInput").ap(),
        'blkid': nc.dram_tensor("blkid", [1, 1], mybir.dt.int32,
                                kind="ExternalInput").ap(),
        'coefr': nc.dram_tensor("coefr", [128, NSPEC * W], f16,
                                kind="ExternalInput").ap(),
        'biasd': nc.dram_tensor("biasd", [O, 1], f32,
                                kind="ExternalInput").ap(),
        'betad': nc.dram_tensor("betad", [128, NROW], f32,
                                kind="ExternalInput").ap(),
        'out': nc.dram_tensor("out", [NROW, O, W], f16,
                              kind="ExternalOutput").ap(),
    }
    with tile.TileContext(nc) as tc:
        _emit_kernel(tc, aps, blocks, plans, betas, spec_ev)
    nc.finalize()

    _CACHE['prog'] = (nc, tt, blocks, E, plans, scales, kmaps, betas)
    return _CACHE['prog']


def _core_inputs(x, weight, bias, tt, blocks, E, scales, kmaps, betas):
    """Assemble per-core in_maps. Core c = batch (c // 4), band (c % 4)."""
    w3 = weight.reshape(O, C, K).astype(np.float64)
    w2d = np.empty((128, K, O), np.float64)
    w2d[:C] = w3.transpose(1, 2, 0)
    w2d[C:] = w3.transpose(1, 2, 0)
    biasd = np.ascontiguousarray(bias.reshape(O, 1).astype(np.float32))

    lts_on = np.zeros((128, NSPEC * O), np.float16)
    for jj in range(NSPEC):
        lts_on[:C, jj * O:(jj + 1) * O] = w2d[:C, 1, :].astype(np.float16)
    lts_off = np.zeros((128, NSPEC * O), np.float16)

    Gam = tt['Gam'].astype(np.float16)
    coef_on = np.ascontiguousarray(
        np.broadcast_to(Gam[:, None, :], (NSPEC, 128, W))
        .transpose(1, 0, 2).reshape(128, NSPEC * W))
    coef_off = np.zeros((128, NSPEC * W), np.float16)

    # per-band stationary tables: lt[lh, p, ti*O+o] = w2d[p, k(ti), o]*sc
    lt_blk = []
    beta_blk = []
    bias64 = bias.astype(np.float64)
    for blk in range(4):
        ltv = np.zeros((NROW, 128, NTAP * O), np.float16)
        betad = np.zeros((128, NROW), np.float32)
        sc = scales[blk]
        for lh in range(NROW):
            for ti, k in enumerate(kmaps[blk][lh]):
                blkw = (w2d[:, k, :] * sc[lh, ti][:, None]).astype(np.float16)
                ltv[lh, :, ti * O:(ti + 1) * O] = blkw
            car = betas[blk][lh]
            if car is not None:
                # beta solves ltq^T beta = bias against the fp16-quantized
                # stationary of the carrier tap (bias then rides the carrier
                # z-build's activation for free)
                ti = car[1]
                ltq = ltv[lh, :, ti * O:(ti + 1) * O].astype(np.float64)
                beta, *_ = np.linalg.lstsq(ltq.T, bias64, rcond=None)
                b32 = beta.astype(np.float32)
                resid = np.abs(ltq.T @ b32.astype(np.float64) - bias64).max()
                assert resid < 5e-3, (blk, lh, resid)
                betad[:, lh] = b32
        lt_blk.append(np.ascontiguousarray(ltv))
        beta_blk.append(betad)

    in_maps = []
    for cid in range(NCORES):
        b, blk = cid // 4, cid % 4
        xz = np.concatenate([x[b], np.zeros((C, 1, W), x.dtype)], axis=1)
        xz = xz.astype(np.float16)
        rows = np.asarray(blocks[blk]['events'], np.int64)
        pair_idx = np.stack([rows, rows + 1], axis=1)       # [E, 2]
        xbv = xz[:, pair_idx, :]                            # [C, E, 2, W]
        xbv = np.ascontiguousarray(xbv.transpose(1, 2, 0, 3))  # [E,2,C,W]
        in_maps.append({
            'xb': xbv,
            'lt': lt_blk[blk],
            'lts': lts_on if blk == 0 else lts_off,
            'blkid': np.array([[blk]], np.int32),
            'coefr': coef_on if blk == 0 else coef_off,
            'biasd': biasd,
            'betad': beta_blk[blk],
        })
    return in_maps


def kernel(x, weight, bias):
    from concourse.bass_utils import run_bass_kernel_spmd
    x = np.asarray(x, dtype=np.float32)
    weight = np.asarray(weight, dtype=np.float32)
    bias = np.asarray(bias, dtype=np.float32)

    nc, tt, blocks, E, plans, scales, kmaps, betas = _get_compiled()
    in_maps = _core_inputs(x, weight, bias, tt, blocks, E, scales, kmaps,
                           betas)
    res = run_bass_kernel_spmd(nc, in_maps, core_ids=list(range(NCORES)))

    out = np.empty((B, O, H, W), np.float32)
    for cid in range(NCORES):
        b, blk = cid // 4, cid % 4
        oc = res.results[cid]['out'].astype(np.float32)     # [NROW, O, W]
        out[b, :, blk * NROW:(blk + 1) * NROW, :] = oc.transpose(1, 0, 2)
    return out


# revision 39
# speedup vs baseline: 1.0721x; 1.0721x over previous
"""Trainium2 Bass kernel for nn_EquiConv2d (equirectangular deformable conv).

Structure (v2 — x-interp prefused on vector engines):
  * off_y is longitude-invariant: each (tap k, row h) samples a fixed input
    row-pair (iy0, iy0+1) with constant y-fractions -> row-pair tiles F
    ([128 = 2rows x 64ch, 1024 = row duplicated for circular reads]).
  * off_x is longitude-invariant up to the 2*pi wrap: sampling along a row is
    a circular shift s0(k,h) plus a constant x-fraction fr.
  * v1 used 2 matmuls per tap (x0/x1 corners, scales folded in stationary).
    v2 prefuses the x-interp on DVE/Pool/Act:
        z = win_a + c*win_b   (c = min(fr,1-fr)/max(..) <= 1)
    so each tap is ONE matmul [128-contraction, 512-free] with the max corner
    weight folded into the per-(row,tap) stationary -> 9 matmuls/row.
  * Seam zero-padding semantics (reference treats x outside [0,512) as zero):
    a "G-type" window equals the F window except column 512 of F must read 0.
    Instead of staging a second zeroed tile: prefused taps patch the single
    affected z column ([128,1] copy / tensor_scalar); single-slot taps split
    their matmul around the affected output column (PSUM accumulation makes
    the skipped column exactly the required zero contribution).
  * fp32 oddities: tap (7,255) dead; tap (1,1) antipode handled by 3 extra
    data-driven matmul slots (active only on cores owning global row 1).

Sharding: 8 cores = 2 batches x 4 bands of 64 output rows.
"""

import math

import numpy as np

# ----------------------------------------------------------------------------
# problem constants
B, C, H, W = 2, 64, 256, 512
O, KH, KW = 64, 3, 3
K = KH * KW
NCORES = 8
NROW = 64            # output rows per core
MAXZ = 5             # prefused doubles per row (engine capacity bound)
NACT = 2             # prefused builds whose mult half runs on Act
NTAP = 10            # stationary slots per row (9 taps + un-prefused extras)
NSPEC = 3            # special (antipode) slots, accumulated into local row 1
RING = 16            # staged row-pair ring slots
PF = 4               # staging prefetch lead (rows)
SLOTW = 1024         # F columns per ring slot (row-pair duplicated)
SKIP_TOL = 1e-4      # drop corner slots with |weight| below this

_CACHE = {}


# ----------------------------------------------------------------------------
# host-side geometry tables (must replicate reference fp32 semantics exactly)

def _compute_offsets_jax():
    """Bit-exact replica of reference.equi_offsets on jax CPU."""
    import jax
    import jax.numpy as jnp
    cpu = jax.devices("cpu")[0]
    with jax.default_device(cpu):
        dtype = jnp.float32
        pano_H, pano_W, kH, kW = H, W, KH, KW
        Kk = kH * kW
        u = jnp.arange(pano_W, dtype=dtype)
        v = jnp.arange(pano_H, dtype=dtype)
        phi = (u - pano_W / 2.0) / pano_W * (2.0 * math.pi)
        theta = -(v - pano_H / 2.0) / pano_H * math.pi
        cp, sp = jnp.cos(phi), jnp.sin(phi)
        z, one = jnp.zeros_like(cp), jnp.ones_like(cp)
        Ry = jnp.stack([jnp.stack([cp, z, sp], -1),
                        jnp.stack([z, one, z], -1),
                        jnp.stack([-sp, z, cp], -1)], -2)
        ct, st = jnp.cos(theta), jnp.sin(theta)
        zh, oh = jnp.zeros_like(ct), jnp.ones_like(ct)
        Rx = jnp.stack([jnp.stack([oh, zh, zh], -1),
                        jnp.stack([zh, ct, -st], -1),
                        jnp.stack([zh, st, ct], -1)], -2)
        ROT = jnp.einsum('wij,hjk->hwik', Ry, Rx)
        fov_w = kW * (2.0 * math.pi / pano_W)
        focal = (kW / 2.0) / math.tan(fov_w / 2.0)
        hg = (jnp.arange(kH, dtype=dtype)[:, None] + 0.5 - kH / 2.0)
        wg = (jnp.arange(kW, dtype=dtype)[None, :] + 0.5 - kW / 2.0)
        hg = jnp.broadcast_to(hg, (kH, kW)).reshape(Kk)
        wg = jnp.broadcast_to(wg, (kH, kW)).reshape(Kk)
        rays0 = jnp.stack([wg / focal, hg / focal, jnp.ones(Kk, dtype)], 0)
        rays0 = rays0 / jnp.linalg.norm(rays0, axis=0, keepdims=True)
        rays = jnp.einsum('hwik,kn->hwin', ROT, rays0)
        phi2 = jnp.arctan2(rays[..., 0, :], rays[..., 2, :])
        th2 = jnp.arcsin(jnp.clip(rays[..., 1, :], -1.0, 1.0))
        x = pano_W / (2.0 * math.pi) * phi2 + pano_W / 2.0
        y = pano_H / math.pi * th2 + pano_H / 2.0
        off_x = x - (wg[None, None, :] + u[None, :, None])
        off_y = y - (hg[None, None, :] + v[:, None, None])
        return (np.asarray(jnp.transpose(off_y, (2, 0, 1))),
                np.asarray(jnp.transpose(off_x, (2, 0, 1))))


def _build_tap_tables():
    off_y, off_x = _compute_offsets_jax()
    ky = np.repeat(np.arange(KH), KW).astype(np.float32)
    kx = np.tile(np.arange(KW), KH).astype(np.float32)
    base_x = (np.arange(W, dtype=np.float32) - np.float32(1))
    base_y = (np.arange(H, dtype=np.float32) - np.float32(1))
    px = (base_x[None, None, :] + kx[:, None, None] + off_x).astype(np.float32)
    py = (base_y[None, :, None] + ky[:, None, None] + off_y).astype(np.float32)
    pyc = py[:, :, 0]
    assert np.all(py == pyc[:, :, None]), "off_y not longitude-invariant"

    iy0 = np.floor(pyc).astype(np.int64)
    wy1 = (pyc - np.floor(pyc)).astype(np.float64)
    v0 = (iy0 >= 0) & (iy0 < H)
    v1 = (iy0 + 1 >= 0) & (iy0 + 1 < H)
    cy0 = np.where(v0, 1.0 - wy1, 0.0)
    cy1 = np.where(v1, wy1, 0.0)

    Draw = np.mod((px.astype(np.float64) - np.arange(W)[None, None, :]), 512.0)
    ang = Draw / 512.0 * 2 * np.pi
    mean = np.mod(np.angle(np.exp(1j * ang).mean(axis=2)) / (2 * np.pi) * 512.0,
                  512.0)
    resid = np.mod(Draw - mean[:, :, None] + 256.0, 512.0) - 256.0
    D = mean + np.median(resid, axis=2)
    s0 = np.mod(np.floor(D), 512).astype(np.int64)
    frac = D - np.floor(D)

    special = np.zeros((K, H), dtype=bool)
    special[1, 1] = True
    dead = (cy0 == 0.0) & (cy1 == 0.0)

    Ddev = np.abs(np.mod(Draw - D[:, :, None] + 256.0, 512.0) - 256.0)
    dev = Ddev.max(axis=2)
    bad = (dev > 5e-4) & ~special & ~dead
    assert not bad.any(), f"unrepresentable taps: {np.argwhere(bad)}"

    def ref_coefs(p):
        x0 = math.floor(p)
        fr = p - x0
        out = {}
        for ix, wt in ((x0, 1.0 - fr), (x0 + 1, fr)):
            if 0 <= ix < W and wt != 0.0:
                out[ix] = out.get(ix, 0.0) + wt
        return out

    # seam variant selection: decided by the exact fp32 px at the wrap column
    slot0_useG = np.zeros((K, H), dtype=bool)
    slot1_useF = np.zeros((K, H), dtype=bool)
    for k in range(K):
        for h in range(H):
            if special[k, h] or dead[k, h]:
                continue
            s = int(s0[k, h]); fr = frac[k, h]
            if s >= 1:
                w0 = (512 - s) % 512
                rc = ref_coefs(float(px[k, h, w0]))
                slot0_useG[k, h] = (abs(rc.get(0, 0.0))
                                    < abs(rc.get(0, 0.0) - (1 - fr)))
            w1 = (511 - s) % 512
            rc = ref_coefs(float(px[k, h, w1]))
            slot1_useF[k, h] = (abs(rc.get(0, 0.0) - fr)
                                < abs(rc.get(0, 0.0)))

    # special tap (1,1): per-column coefficients on F offsets 255..257
    pxs = px[1, 1, :].astype(np.float64)
    Gam = np.zeros((3, W), dtype=np.float64)
    for w in range(W):
        p = pxs[w]
        x0 = math.floor(p)
        fr = p - x0
        for ix, wt in ((x0, 1.0 - fr), (x0 + 1, fr)):
            if 0 <= ix < W and wt != 0.0:
                found = False
                for jj in range(3):
                    if (255 + jj + w) % 512 == ix % 512:
                        Gam[jj, w] += wt
                        found = True
                        break
                assert found, (w, p, ix)

    return dict(iy0=iy0, cy0=cy0, cy1=cy1, s0=s0, frac=frac,
                slot0_useG=slot0_useG, slot1_useF=slot1_useF,
                special=special, dead=dead, Gam=Gam)


# ----------------------------------------------------------------------------
# uniform SPMD schedule (events = staged row-pairs per band)

def _build_schedule(tt):
    blocks = []
    for blk in range(4):
        h0 = blk * NROW
        ev_of, events, first_use = {}, [], []
        need = np.zeros((NROW, K), np.int64)
        for lh in range(NROW):
            for k in range(K):
                r = int(np.clip(tt['iy0'][k, h0 + lh], 0, 255))
                if r not in ev_of:
                    ev_of[r] = len(events)
                    events.append(r)
                    first_use.append(lh)
                need[lh, k] = ev_of[r]
        blocks.append(dict(events=events, first_use=first_use, need=need))

    E = max(len(b['events']) for b in blocks)
    for b in blocks:
        while len(b['events']) < E:
            b['events'].append(b['events'][-1])
    return blocks, E


# ----------------------------------------------------------------------------
# per-row tap plan: windows, prefusion, patches, splits, engine assignment

def _build_plan(tt, blocks):
    """plans[blk][lh] = ordered list of tap items (full-width first):
       ('z', ev, f0a, f0b, c, patch_a, patch_b)   prefused double
       ('s', ev, f0, ws)                          single; ws=None -> full
    At most MAXZ doubles per row are prefused (engine capacity); the rest
    (those with the most seam patches, which become ~free PE splits) are
    emitted as two single slots.
    Also returns scales[blk][lh, ti, :] (stationary scale [128]) and
    kmaps[blk][lh][ti] (source tap k per stationary slot).
    """
    plans, scales, kmaps = [], [], []
    for blk in range(4):
        need = blocks[blk]['need']
        rows, krows = [], []
        sc = np.zeros((NROW, NTAP, 128), np.float64)
        for lh in range(NROW):
            h = blk * NROW + lh
            doubles, singles = [], []
            for k in range(K):
                if tt['dead'][k, h] or tt['special'][k, h]:
                    continue
                ev = int(need[lh, k])
                s = int(tt['s0'][k, h])
                fr = float(tt['frac'][k, h])
                c0, c1 = float(tt['cy0'][k, h]), float(tt['cy1'][k, h])
                f0a, f0b = s, s + 1
                a_fp = bool(tt['slot0_useG'][k, h]) and s >= 1 and f0a >= 1
                b_fp = (not bool(tt['slot1_useF'][k, h])) and f0b >= 1
                e0 = 1.0 - fr >= SKIP_TOL
                e1 = fr >= SKIP_TOL
                if e0 and e1:
                    doubles.append((k, ev, f0a, f0b, fr, a_fp, b_fp, c0, c1))
                elif e0 or e1:
                    wt, f0, fp = ((1.0 - fr), f0a, a_fp) if e0 \
                        else (fr, f0b, b_fp)
                    ws = 512 - f0 if fp else None
                    singles.append(((('s', ev, f0, ws), c0 * wt, c1 * wt), k))
            # prefuse the doubles with the fewest patches; un-prefuse rest.
            # build-engine split: first NACT prefused get Act mults ('a'),
            # the rest DVE ts-mults ('v').  DVE-built z's are emitted first
            # in the matmul order (ready earliest).
            doubles.sort(key=lambda d: int(d[5]) + int(d[6]))
            za, zv, zrest = [], [], []
            for di, (k, ev, f0a, f0b, fr, a_fp, b_fp, c0, c1) in \
                    enumerate(doubles):
                if di < MAXZ:
                    if fr <= 0.5:
                        aw, cc = 1.0 - fr, fr / (1.0 - fr)
                        wa_f0, wb_f0, wa_fp, wb_fp = f0a, f0b, a_fp, b_fp
                    else:
                        aw, cc = fr, (1.0 - fr) / fr
                        wa_f0, wb_f0, wa_fp, wb_fp = f0b, f0a, b_fp, a_fp
                    pa = 512 - wa_f0 if wa_fp else None
                    pb = 512 - wb_f0 if wb_fp else None
                    eng = 'a' if di < NACT else 'v'
                    it = ((('z', ev, wa_f0, wb_f0, cc, pa, pb, eng),
                           c0 * aw, c1 * aw), k)
                    (za if eng == 'a' else zv).append(it)
                else:
                    for wt, f0, fp in (((1.0 - fr), f0a, a_fp),
                                       (fr, f0b, b_fp)):
                        ws = 512 - f0 if fp else None
                        it = ((('s', ev, f0, ws), c0 * wt, c1 * wt), k)
                        (zrest if ws is not None else singles).append(it)
            full = zv + za + [it for it in singles if it[0][0][3] is None]
            rest = zrest + [it for it in singles if it[0][0][3] is not None]
            assert full, (blk, lh)
            items = full + rest
            assert len(items) <= NTAP, (blk, lh, len(items))
            row, krow = [], []
            for ti, ((it, s0c, s1c), k) in enumerate(items):
                sc[lh, ti, :64] = s0c
                sc[lh, ti, 64:] = s1c
                row.append(it)
                krow.append(k)
            rows.append(row)
            krows.append(krow)
        plans.append(rows)
        scales.append(sc)
        kmaps.append(krows)

    # beta bias carrier: per row, the first Act-built prefused double.
    # Its activation folds the per-partition beta (solving lt_tap^T beta =
    # bias) in for free; its seam patches become beta-aware [128,1] ops.
    betas = []
    for blk in range(4):
        bt = []
        for lh in range(NROW):
            car = None
            for ti, it in enumerate(plans[blk][lh]):
                if it[0] == 'z' and it[7] == 'a':
                    car = (0, ti)
                    break
            bt.append(car)
        betas.append(bt)
    return plans, scales, kmaps, betas


# ----------------------------------------------------------------------------
# device program

def _emit_section(tc, aps, tiles, pools, blkinfo, plan, betas_row,
                  special_row, shared_dmas):
    """Emit one per-band section (all-static APs)."""
    import concourse.mybir as mybir
    import bass_rust
    nc = tc.nc
    f16 = mybir.dt.float16
    f32 = mybir.dt.float32
    AL = mybir.AluOpType
    ID = mybir.ActivationFunctionType.Identity
    buf, coeft, biast, ltst, betat = tiles
    psp, ltp, zp, zmp, spzp, outp = pools
    xb, outd, lt = aps['xb'], aps['out'], aps['lt']
    first_use = blkinfo['first_use']

    cum = [int(np.searchsorted(np.asarray(first_use), lh, 'right'))
           for lh in range(NROW)]
    tgt = [cum[min(lh + PF, NROW - 1)] for lh in range(NROW)]

    # ring-overwrite feasibility
    E_j = len(first_use)
    ls = [NROW] * E_j
    for e in range(E_j):
        for lh in range(NROW):
            if tgt[lh] > e:
                ls[e] = lh
                break
    lastuse = {}
    need = blkinfo['need']
    for lh in range(NROW):
        for k in range(K):
            lastuse[int(need[lh, k])] = lh
    for e in range(RING, E_j):
        if e - RING in lastuse:
            assert lastuse[e - RING] < ls[e], (e,)

    def stage(e):
        base = (e % RING) * SLOTW
        src = xb[e].rearrange("p c w -> (p c) w")
        nc.sync.dma_start(buf[:, base:base + W], src)
        nc.sync.dma_start(buf[:, base + W:base + 2 * W], src)

    def build_row(lh):
        """z-builds for row lh: mult half on Act ('a' builds, activation
        ~700ns, the carrier folds the per-partition beta bias in for free)
        or DVE ts ('v' builds, ~170ns); adds on DVE, PAIRED two-at-a-time
        via a strided 3-dim in1 AP over buf.  Each pair couples one Act and
        one DVE mult so a slow Act never gates two z's.  Seam patches: ts-
        type on DVE, plain copies on the otherwise-idle Pool."""
        zv = [(ti, it) for ti, it in enumerate(plan[lh])
              if it[0] == 'z' and it[7] == 'v']
        za = [(ti, it) for ti, it in enumerate(plan[lh])
              if it[0] == 'z' and it[7] == 'a']
        carrier = betas_row[lh]
        zs = {}
        # pair one 'a' with one 'v' when possible
        grps = []
        while za or zv:
            g = []
            if za:
                g.append(za.pop(0))
            if zv:
                g.append(zv.pop(0))
            grps.append(g)

        def offa(entry):
            it = entry[1]
            return (it[1] % RING) * SLOTW + it[2]

        for p, grp in enumerate(grps):
            if len(grp) == 2 and offa(grp[0]) > offa(grp[1]):
                grp = [grp[1], grp[0]]
            zpt = zp.tile([128, 2 * W], f16, tag=f"z{p}")
            zmt = zmp.tile([128, 2 * W], f16, tag=f"zm{p}")
            for h, (ti, it) in enumerate(grp):
                _, ev, fa, fb, cc, pa, pb, eng = it
                base = (ev % RING) * SLOTW
                winb = buf[:, base + fb:base + fb + W]
                zmh = zmt[:, h * W:(h + 1) * W]
                if eng == 'a':
                    barg = betat[:, lh:lh + 1] \
                        if (carrier is not None and carrier[1] == ti) else 0.0
                    nc.scalar.activation(zmh, winb, ID, bias=barg,
                                         scale=float(cc))
                else:
                    nc.vector.tensor_scalar(zmh, winb, float(cc), None,
                                            AL.mult)
            if len(grp) == 2 and offa(grp[1]) != offa(grp[0]):
                o0 = offa(grp[0])
                d = offa(grp[1]) - o0
                pair = bass_rust.AP(buf.tensor, buf.offset + o0,
                                    [list(buf.ap[0]), [d, 2], [1, W]])
                zm3 = zmt.rearrange("p (t w) -> p t w", t=2)
                z3 = zpt.rearrange("p (t w) -> p t w", t=2)
                nc.vector.tensor_tensor(z3, zm3, pair, AL.add)
            else:
                for h, (ti, it) in enumerate(grp):
                    base = (it[1] % RING) * SLOTW
                    wina = buf[:, base + it[2]:base + it[2] + W]
                    nc.vector.tensor_tensor(zpt[:, h * W:(h + 1) * W],
                                            zmt[:, h * W:(h + 1) * W],
                                            wina, AL.add)
            for h, (ti, it) in enumerate(grp):
                _, ev, fa, fb, cc, pa, pb, eng = it
                base = (ev % RING) * SLOTW
                iscar = carrier is not None and carrier[1] == ti
                bcol = betat[:, lh:lh + 1]
                if pa is not None:
                    # win_a zero at col pa: z[:,pa] = c*win_b[:,pa] (+beta)
                    if iscar:
                        nc.vector.scalar_tensor_tensor(
                            zpt[:, h * W + pa:h * W + pa + 1],
                            buf[:, base + fb + pa:base + fb + pa + 1],
                            float(cc), bcol, AL.mult, AL.add)
                    else:
                        nc.vector.tensor_scalar(
                            zpt[:, h * W + pa:h * W + pa + 1],
                            buf[:, base + fb + pa:base + fb + pa + 1],
                            float(cc), None, AL.mult)
                if pb is not None:
                    # win_b zero at col pb: z[:,pb] = win_a[:,pb] (+beta)
                    if iscar:
                        nc.vector.tensor_tensor(
                            zpt[:, h * W + pb:h * W + pb + 1],
                            buf[:, base + fa + pb:base + fa + pb + 1],
                            bcol, AL.add)
                    else:
                        nc.gpsimd.tensor_copy(
                            zpt[:, h * W + pb:h * W + pb + 1],
                            buf[:, base + fa + pb:base + fa + pb + 1])
                zs[ti] = zpt[:, h * W:(h + 1) * W]
        return zs

    staged = 0
    zs_d = {}
    for lh in range(NROW):
        while staged < tgt[lh]:
            stage(staged)
            staged += 1
        ltt = ltp.tile([128, NTAP * O], f16, tag="ltt")
        half = NTAP * O // 2
        nc.sync.dma_start(ltt[:, :half], lt[lh][:, :half])
        nc.sync.dma_start(ltt[:, half:], lt[lh][:, half:])
        if lh == 0:
            shared_dmas()
            zs_d[0] = build_row(0)
            if NROW > 1:
                zs_d[1] = build_row(1)
        ps = psp.tile([O, W], f32, tag="ps")

        items = plan[lh]
        # depth-2 software pipeline: z tiles for rows lh..lh+2 are built
        # while the PE works rows lh-2..lh (keeps Act/DVE — and Act's
        # in-order queue behind the out copy — off the PE critical path)
        if lh + 2 < NROW:
            zs_d[lh + 2] = build_row(lh + 2)
        zs = zs_d.pop(lh)

        # matmuls: count instructions first for start/stop flags
        nmm = 0
        for it in items:
            if it[0] == 'z':
                nmm += 1
            else:
                ws = it[3]
                if ws is None:
                    nmm += 1
                else:
                    nmm += int(ws > 0) + int(ws < 511)
        nmm += NSPEC if special_row == lh else 0

        mi = 0
        for ti, it in enumerate(items):
            lts_ap = ltt[:, ti * O:(ti + 1) * O]
            if it[0] == 'z':
                nc.tensor.matmul(ps, lts_ap, zs[ti],
                                 start=(mi == 0), stop=(mi == nmm - 1))
                mi += 1
            else:
                _, ev, f0, ws = it
                base = (ev % RING) * SLOTW
                win = buf[:, base + f0:base + f0 + W]
                if ws is None:
                    nc.tensor.matmul(ps, lts_ap, win,
                                     start=(mi == 0), stop=(mi == nmm - 1))
                    mi += 1
                else:
                    if ws > 0:
                        nc.tensor.matmul(ps[:, 0:ws], lts_ap, win[:, 0:ws],
                                         start=(mi == 0),
                                         stop=(mi == nmm - 1))
                        mi += 1
                    if ws < 511:
                        nc.tensor.matmul(ps[:, ws + 1:W], lts_ap,
                                         win[:, ws + 1:W],
                                         start=(mi == 0),
                                         stop=(mi == nmm - 1))
                        mi += 1

        if special_row == lh:
            sbase = (aps['spec_ev'] % RING) * SLOTW
            for jj in range(NSPEC):
                zt = spzp.tile([128, W], f16, tag="spz")
                nc.vector.tensor_tensor(
                    zt, buf[:, sbase + 255 + jj:sbase + 255 + jj + W],
                    coeft[:, jj * W:(jj + 1) * W], AL.mult)
                nc.tensor.matmul(ps, ltst[:, jj * O:(jj + 1) * O], zt,
                                 start=False, stop=(mi == nmm - 1))
                mi += 1

        ot = outp.tile([O, W], f16, tag="out")
        if betas_row[lh] is not None:
            # bias was injected via the carrier z-build; plain Act copy
            # (GPSIMD cannot read PSUM, and DMA-from-PSUM is unsupported)
            nc.scalar.copy(ot, ps)
        else:
            nc.scalar.activation(ot, ps, ID, bias=biast, scale=1.0)
        nc.sync.dma_start(outd[lh], ot)


def _emit_kernel(tc, aps, blocks, plans, betas, spec_ev):
    import concourse.mybir as mybir
    nc = tc.nc
    f16 = mybir.dt.float16
    f32 = mybir.dt.float32

    with tc.tile_pool(name="bigp", bufs=1) as bigp, \
         tc.tile_pool(name="ltp", bufs=6) as ltp, \
         tc.tile_pool(name="zp", bufs=5) as zp, \
         tc.tile_pool(name="zmp", bufs=5) as zmp, \
         tc.tile_pool(name="spzp", bufs=3) as spzp, \
         tc.tile_pool(name="psp", bufs=6, space="PSUM") as psp, \
         tc.tile_pool(name="outp", bufs=10) as outp:

        buf = bigp.tile([128, RING * SLOTW], f16)
        coeft = bigp.tile([128, NSPEC * W], f16)
        biast = bigp.tile([O, 1], f32)
        ltst = bigp.tile([128, NSPEC * O], f16)
        betat = bigp.tile([128, NROW], f32)

        blkv = nc.values_load(aps['blkid'][0:1, 0:1],
                              min_val=0, max_val=3,
                              skip_runtime_bounds_check=True)

        def shared_dmas():
            # emitted inside each band section AFTER the initial event
            # staging + first lt row so those win the queue-dispatch race
            nc.sync.dma_start(betat, aps['betad'])
            nc.sync.dma_start(biast, aps['biasd'])
            nc.sync.dma_start(ltst, aps['lts'])
            nc.sync.dma_start(coeft, aps['coefr'])

        tiles = (buf, coeft, biast, ltst, betat)
        pools = (psp, ltp, zp, zmp, spzp, outp)
        for j in range(4):
            with tc.If(blkv == j):
                aps['spec_ev'] = spec_ev
                _emit_section(tc, aps, tiles, pools, blocks[j], plans[j],
                              betas[j], special_row=(1 if j == 0 else -1),
                              shared_dmas=shared_dmas)


def _get_compiled():
    """Build tables, schedule, and the Bass program once."""
    if 'prog' in _CACHE:
        return _CACHE['prog']
    import concourse.mybir as mybir
    import concourse.tile as tile
    from concourse import bacc

    tt = _build_tap_tables()
    blocks, E = _build_schedule(tt)
    plans, scales, kmaps, betas = _build_plan(tt, blocks)
    spec_ev = int(blocks[0]['need'][1, 1])

    f16 = mybir.dt.float16
    f32 = mybir.dt.float32
    nc = bacc.Bacc("TRN2", target_bir_lowering=False, debug=False,
                   num_devices=NCORES)
    aps = {
        'xb': nc.dram_tensor("xb", [E, 2, C, W], f16,
                             kind="ExternalInput").ap(),
        'lt': nc.dram_tensor("lt", [NROW, 128, NTAP * O], f16,
                             kind="ExternalInput").ap(),
        'lts': nc.dram_tensor("lts", [128, NSPEC * O], f16,
                              kind="ExternalInput").ap(),
        'blkid': nc.dram_tensor("blkid", [1, 1], mybir.dt.int32,
                                kind="ExternalInput").ap(),
        'coefr': nc.dram_tensor("coefr", [128, NSPEC * W], f16,
                                kind="ExternalInput").ap(),
        'biasd': nc.dram_tensor("biasd", [O, 1], f32,
                                kind="ExternalInput").ap(),
        'betad': nc.dram_tensor("betad", [128, NROW], f32,
                                kind="ExternalInput").ap(),
        'out': nc.dram_tensor("out", [NROW, O, W], f16,
                              kind="ExternalOutput").ap(),
    }
    with tile.TileContext(nc) as tc:
        _emit_kernel(tc, aps, blocks, plans, betas, spec_ev)
    nc.finalize()

    _CACHE['prog'] = (nc, tt, blocks, E, plans, scales, kmaps, betas)
    return _CACHE['prog']


def _core_inputs(x, weight, bias, tt, blocks, E, scales, kmaps, betas):
    """Assemble per-core in_maps. Core c = batch (c // 4), band (c % 4)."""
    w3 = weight.reshape(O, C, K).astype(np.float64)
    w2d = np.empty((128, K, O), np.float64)
    w2d[:C] = w3.transpose(1, 2, 0)
    w2d[C:] = w3.transpose(1, 2, 0)
    biasd = np.ascontiguousarray(bias.reshape(O, 1).astype(np.float32))

    lts_on = np.zeros((128, NSPEC * O), np.float16)
    for jj in range(NSPEC):
        lts_on[:C, jj * O:(jj + 1) * O] = w2d[:C, 1, :].astype(np.float16)
    lts_off = np.zeros((128, NSPEC * O), np.float16)

    Gam = tt['Gam'].astype(np.float16)
    coef_on = np.ascontiguousarray(
        np.broadcast_to(Gam[:, None, :], (NSPEC, 128, W))
        .transpose(1, 0, 2).reshape(128, NSPEC * W))
    coef_off = np.zeros((128, NSPEC * W), np.float16)

    # per-band stationary tables: lt[lh, p, ti*O+o] = w2d[p, k(ti), o]*sc
    lt_blk = []
    beta_blk = []
    bias64 = bias.astype(np.float64)
    for blk in range(4):
        ltv = np.zeros((NROW, 128, NTAP * O), np.float16)
        betad = np.zeros((128, NROW), np.float32)
        sc = scales[blk]
        for lh in range(NROW):
            for ti, k in enumerate(kmaps[blk][lh]):
                blkw = (w2d[:, k, :] * sc[lh, ti][:, None]).astype(np.float16)
                ltv[lh, :, ti * O:(ti + 1) * O] = blkw
            car = betas[blk][lh]
            if car is not None:
                # beta solves ltq^T beta = bias against the fp16-quantized
                # stationary of the carrier tap (bias then rides the carrier
                # z-build's activation for free)
                ti = car[1]
                ltq = ltv[lh, :, ti * O:(ti + 1) * O].astype(np.float64)
                beta, *_ = np.linalg.lstsq(ltq.T, bias64, rcond=None)
                b32 = beta.astype(np.float32)
                resid = np.abs(ltq.T @ b32.astype(np.float64) - bias64).max()
                assert resid < 5e-3, (blk, lh, resid)
                betad[:, lh] = b32
        lt_blk.append(np.ascontiguousarray(ltv))
        beta_blk.append(betad)

    in_maps = []
    for cid in range(NCORES):
        b, blk = cid // 4, cid % 4
        xz = np.concatenate([x[b], np.zeros((C, 1, W), x.dtype)], axis=1)
        xz = xz.astype(np.float16)
        rows = np.asarray(blocks[blk]['events'], np.int64)
        pair_idx = np.stack([rows, rows + 1], axis=1)       # [E, 2]
        xbv = xz[:, pair_idx, :]                            # [C, E, 2, W]
        xbv = np.ascontiguousarray(xbv.transpose(1, 2, 0, 3))  # [E,2,C,W]
        in_maps.append({
            'xb': xbv,
            'lt': lt_blk[blk],
            'lts': lts_on if blk == 0 else lts_off,
            'blkid': np.array([[blk]], np.int32),
            'coefr': coef_on if blk == 0 else coef_off,
            'biasd': biasd,
            'betad': beta_blk[blk],
        })
    return in_maps


def kernel(x, weight, bias):
    from concourse.bass_utils import run_bass_kernel_spmd
    x = np.asarray(x, dtype=np.float32)
    weight = np.asarray(weight, dtype=np.float32)
    bias = np.asarray(bias, dtype=np.float32)

    nc, tt, blocks, E, plans, scales, kmaps, betas = _get_compiled()
    in_maps = _core_inputs(x, weight, bias, tt, blocks, E, scales, kmaps,
                           betas)
    res = run_bass_kernel_spmd(nc, in_maps, core_ids=list(range(NCORES)))

    out = np.empty((B, O, H, W), np.float32)
    for cid in range(NCORES):
        b, blk = cid // 4, cid % 4
        oc = res.results[cid]['out'].astype(np.float32)     # [NROW, O, W]
        out[b, :, blk * NROW:(blk + 1) * NROW, :] = oc.transpose(1, 0, 2)
    return out
